# revision 9
# baseline (speedup 1.0000x reference)
"""Block-sparse attention backward pass on 8 TRN2 NeuronCores.

Sharding: head-parallel - 16 heads / 8 cores = 2 heads per core. The
block mask is shared by all heads, so every core runs the SAME program
(true SPMD); only the data shards differ. All dQ/dK/dV accumulation is
local to a head shard: no cross-core communication.

Math per active (i, j) block pair (local per-block softmax):
  S_ij = q_i k_j^T                  (PE, row-group 0: stationary [qT;dOT])
  dA_ij = dO_i v_j^T                (PE, row-group 1, concurrent)
  U = exp(S * scale)                (ACT, chunked)
  W = U o dA                        (DVE, chunked, 1x: dA is f32 PSUM)
  [l | rs] = rowsums of [U | W]     (DVE, one chunked grouped reduce, 2x)
  r = 1/l  (approx)                 (DVE RECIPROCAL_APPROX_FAST)
  rd = rs o r                       (DVE, small)
  t = W - U o rd                    (DVE LN_BWD_DX_ANT custom op, 2x, per pair)
  dS = t o r                        (GPSIMD broadcast, chunked)
  dop = dO_i o r                    (GPSIMD broadcast, chunked, from dONp)
  dV^T_j += dop^T U                 (PE col-group 0 of a shared bank)
  dK^T_j += (q*scale)^T dS          (PE col-group 1, same bank)
  dS^T via PE transpose             (PE, identity moving)
  dQ^T_i += (k*scale)^T dS^T        (PE pass 2, col-tiled 2 i's per bank)

dV/dK/dQ are produced transposed [64, N]; the host transposes back.

PSUM budget (8 banks): s_ps 2x[128,1024]f32 (4) + da_ps 1x[128,1024]f32
(2) + dst 1x[128,1024]bf16 (1) + dvk 1x[128,128]f32 (1). The dvk bank
holds dV^T (rows 0-63) and dK^T (rows 64-127): exactly one start=True
per j (first dV matmul) clears the bank; every other matmul uses
start=False and lands on has_written=0 regions (overwrite) or
accumulates - per-element semantics make the interleave correct.
"""

import sys

sys.path.insert(0, "/opt/trn_rl_repo")

import numpy as np
import ml_dtypes

import concourse.bass as bass
import concourse.mybir as mybir
import concourse.tile as tile
from concourse import bacc
from concourse.bass_utils import run_bass_kernel_spmd
from concourse.masks import make_identity
from concourse.dve_ops import LN_BWD_DX_ANT

BF16 = mybir.dt.bfloat16
F32 = mybir.dt.float32
OP = mybir.AluOpType
ACTF = mybir.ActivationFunctionType

N, D, H, DK, BLK, T = 2048, 1024, 16, 64, 128, 16
NCORES, HPC = 8, 2  # heads per core
SCALE = float(1.0 / np.sqrt(DK))  # tau=1
CH = 8  # pairs per chunk

_BF = ml_dtypes.bfloat16


def _build(mask_key):
    """Build the SPMD program for one core (2 heads), specialized on the mask."""
    mask = np.array(mask_key, dtype=np.int64).reshape(T, T)
    act_per_j = [[i for i in range(T) if mask[i, j]] for j in range(T)]
    act_per_i = [[j for j in range(T) if mask[i, j]] for i in range(T)]
    npair = int(mask.sum())
    # flat pair stream, j-major; pidx[(i, j)] = position in stream
    pairs = [(i, j) for j in range(T) for i in act_per_j[j]]
    pidx = {p: n for n, p in enumerate(pairs)}
    # chunks of up to CH consecutive pairs
    chunks = [pairs[c:c + CH] for c in range(0, npair, CH)]

    nc = bacc.Bacc("TRN2", target_bir_lowering=False, debug=False)

    # per-head inputs
    qdo = [nc.dram_tensor(f"qdo{h}", [128, N], BF16, kind="ExternalInput")
           for h in range(HPC)]
    kv = [nc.dram_tensor(f"kv{h}", [128, N], BF16, kind="ExternalInput")
          for h in range(HPC)]
    qns = [nc.dram_tensor(f"qns{h}", [128, T * DK], BF16, kind="ExternalInput")
           for h in range(HPC)]
    kns = [nc.dram_tensor(f"kns{h}", [128, T * DK], BF16, kind="ExternalInput")
           for h in range(HPC)]
    donp = [nc.dram_tensor(f"donp{h}", [128, npair * DK], BF16,
                           kind="ExternalInput") for h in range(HPC)]

    # transposed outputs [head, d, N]; host transposes back
    dQo = nc.dram_tensor("dQo", [HPC, DK, N], F32, kind="ExternalOutput")
    dKo = nc.dram_tensor("dKo", [HPC, DK, N], F32, kind="ExternalOutput")
    dVo = nc.dram_tensor("dVo", [HPC, DK, N], F32, kind="ExternalOutput")

    with tile.TileContext(nc) as tc:
        with (
            tc.tile_pool(name="const", bufs=1) as constp,
            tc.tile_pool(name="inp", bufs=1) as inp,
            tc.tile_pool(name="dstore", bufs=1) as dstore,
            tc.tile_pool(name="uw", bufs=2) as uwp,
            tc.tile_pool(name="dsp", bufs=2) as dsp,
            tc.tile_pool(name="stat", bufs=2) as statp,
            tc.tile_pool(name="outsb", bufs=4) as outsb,
        ):
            ident = constp.tile([128, 128], BF16)
            make_identity(nc, ident[:])

            tqdo, tkv, tqns, tkns, tdonp = [], [], [], [], []
            for h in range(HPC):
                tqdo.append(inp.tile([128, N], BF16, name=f"tqdo{h}",
                                     tag=f"qdo{h}"))
                tkv.append(inp.tile([128, N], BF16, name=f"tkv{h}",
                                    tag=f"kv{h}"))
                tqns.append(inp.tile([128, T * DK], BF16, name=f"tqns{h}",
                                     tag=f"qns{h}"))
                tkns.append(inp.tile([128, T * DK], BF16, name=f"tkns{h}",
                                     tag=f"kns{h}"))
                tdonp.append(inp.tile([128, npair * DK], BF16,
                                      name=f"tdonp{h}", tag=f"donp{h}"))
                nc.sync.dma_start(tqdo[h][:], qdo[h][:])
                nc.sync.dma_start(tkv[h][:], kv[h][:])
                nc.sync.dma_start(tqns[h][:], qns[h][:])
                nc.sync.dma_start(tkns[h][:], kns[h][:])
                nc.sync.dma_start(tdonp[h][:], donp[h][:])

            # dS^T of every active pair, per head, bf16
            dstTs = [dstore.tile([128, npair * BLK], BF16, name=f"dstT{h}",
                                 tag=f"dstT{h}") for h in range(HPC)]

            for h in range(HPC):
                dstT = dstTs[h]
                with (
                    tc.tile_pool(name="ps_s", bufs=2, space="PSUM") as ps_s,
                    tc.tile_pool(name="ps_da", bufs=1, space="PSUM") as ps_da,
                    tc.tile_pool(name="ps_dst", bufs=1, space="PSUM") as ps_dst,
                    tc.tile_pool(name="ps_dvk", bufs=1, space="PSUM") as ps_dvk,
                ):
                    dvk = None          # open [128,128] accumulator tile
                    dvk_j = -1          # which j it belongs to
                    dvk_n = 0           # matmuls emitted into it

                    def flush_dvk():
                        nonlocal dvk
                        if dvk is None:
                            return
                        sb = outsb.tile([DK, 2 * BLK], F32, name="dvksb",
                                        tag="dvk")
                        nc.scalar.copy(sb[:], dvk[:])
                        nc.sync.dma_start(
                            dVo[h, :, dvk_j * BLK:(dvk_j + 1) * BLK],
                            sb[:, 0:BLK])
                        nc.sync.dma_start(
                            dKo[h, :, dvk_j * BLK:(dvk_j + 1) * BLK],
                            sb[:, BLK:2 * BLK])
                        dvk = None

                    for chunk in chunks:
                        m = len(chunk)
                        p0 = pidx[chunk[0]]
                        s_ps = ps_s.tile([128, CH * BLK], F32, tag="s")
                        da_ps = ps_da.tile([128, CH * BLK], F32, tag="da")
                        dst_ps = ps_dst.tile([128, CH * BLK], BF16, tag="dst")
                        UW = uwp.tile([128, 2 * CH * BLK], BF16, tag="UW")
                        U = UW[:, :CH * BLK]
                        W = UW[:, CH * BLK:]
                        tS = dsp.tile([128, CH * BLK], BF16, tag="tS")
                        dS = dsp.tile([128, CH * BLK], BF16, tag="dS")
                        dop = dsp.tile([128, CH * DK], BF16, tag="dop")
                        # stats: [l | rs] then r, rd
                        lrs = statp.tile([128, 2 * CH], F32, tag="lrs")
                        rrd = statp.tile([128, 2 * CH], F32, tag="rrd")
                        lt = lrs[:, 0:m]
                        rst = lrs[:, CH:CH + m]
                        rt = rrd[:, 0:m]
                        rdt = rrd[:, CH:CH + m]

                        for x, (i, j) in enumerate(chunk):
                            cs = slice(x * BLK, (x + 1) * BLK)
                            nc.tensor.matmul(
                                s_ps[:, cs],
                                tqdo[h][0:DK, i * BLK:(i + 1) * BLK],
                                tkv[h][0:DK, j * BLK:(j + 1) * BLK],
                                start=True, stop=True)
                            nc.tensor.matmul(
                                da_ps[:, cs],
                                tqdo[h][DK:128, i * BLK:(i + 1) * BLK],
                                tkv[h][DK:128, j * BLK:(j + 1) * BLK],
                                start=True, stop=True)

                        nc.scalar.activation(U[:, :m * BLK], s_ps[:, :m * BLK],
                                             ACTF.Exp, scale=SCALE)
                        nc.vector.tensor_tensor(
                            W[:, :m * BLK], U[:, :m * BLK], da_ps[:, :m * BLK],
                            op=OP.mult)
                        # [l | rs] in one grouped reduce over [U | W]
                        if m == CH:
                            nc.vector.tensor_reduce(
                                lrs[:, 0:2 * CH],
                                UW[:].rearrange("p (g x) -> p g x", x=BLK),
                                axis=mybir.AxisListType.X, op=OP.add)
                        else:
                            nc.vector.tensor_reduce(
                                lrs[:, 0:m],
                                U[:, :m * BLK].rearrange(
                                    "p (g x) -> p g x", x=BLK),
                                axis=mybir.AxisListType.X, op=OP.add)
                            nc.vector.tensor_reduce(
                                lrs[:, CH:CH + m],
                                W[:, :m * BLK].rearrange(
                                    "p (g x) -> p g x", x=BLK),
                                axis=mybir.AxisListType.X, op=OP.add)
                        nc.vector.reciprocal_approx_fast(out=rt, in_=lt)
                        nc.vector.tensor_tensor(rdt, rst, rt, op=OP.mult)

                        # t = W - U o rd   (per pair; custom DVE, all-bf16 2x)
                        for x in range(m):
                            cs = slice(x * BLK, (x + 1) * BLK)
                            nc.vector._custom_dve(
                                LN_BWD_DX_ANT,
                                out=tS[:, cs], in0=W[:, cs], in1=U[:, cs],
                                s0=rrd[:, CH + x:CH + x + 1], s1=0.0, imm2=1.0)
                        # dS = t o r ; dop = dOblk o r   (broadcast, gpsimd)
                        nc.gpsimd.tensor_tensor(
                            dS[:, :m * BLK].rearrange("p (g x) -> p g x", x=BLK),
                            tS[:, :m * BLK].rearrange("p (g x) -> p g x", x=BLK),
                            rrd[:, 0:m][:, :, None].broadcast_to([128, m, BLK]),
                            op=OP.mult)
                        nc.gpsimd.tensor_tensor(
                            dop[:, :m * DK].rearrange("p (g x) -> p g x", x=DK),
                            tdonp[h][:, p0 * DK:(p0 + m) * DK].rearrange(
                                "p (g x) -> p g x", x=DK),
                            rrd[:, 0:m][:, :, None].broadcast_to([128, m, DK]),
                            op=OP.mult)

                        for x, (i, j) in enumerate(chunk):
                            cs = slice(x * BLK, (x + 1) * BLK)
                            if j != dvk_j:
                                flush_dvk()
                                dvk = ps_dvk.tile([DK, 2 * BLK], F32,
                                                  name="dvkps", tag="dvk")
                                dvk_j, dvk_n = j, 0
                                npair_j = len(act_per_j[j])
                            first = dvk_n == 0
                            last = dvk_n == npair_j - 1
                            # dV^T_j += dop_x^T U_x   (bank cols 0-127)
                            nc.tensor.matmul(
                                dvk[:, 0:BLK],
                                dop[:, x * DK:(x + 1) * DK],
                                U[:, cs],
                                start=first, stop=last,
                                skip_group_check=True)
                            # dK^T_j += qns_i^T dS_x  (bank cols 128-255)
                            nc.tensor.matmul(
                                dvk[:, BLK:2 * BLK],
                                tqns[h][:, i * DK:(i + 1) * DK],
                                dS[:, cs],
                                start=False, stop=last,
                                skip_group_check=True)
                            dvk_n += 1
                            # dS^T via PE transpose
                            nc.tensor.transpose(dst_ps[:, cs], dS[:, cs],
                                                ident[:])
                        nc.scalar.copy(dstT[:, p0 * BLK:(p0 + m) * BLK],
                                       dst_ps[:, :m * BLK])
                    flush_dvk()

                # pass 2: dQ^T_i = sum_j kns_j^T dS^T_ij, two i's per bank
                with tc.tile_pool(name="ps_dq", bufs=2, space="PSUM") as ps_dq:
                    ilist = [i for i in range(T) if act_per_i[i]]
                    for g in range(0, len(ilist), 2):
                        ig = ilist[g:g + 2]
                        dq = ps_dq.tile([DK, 2 * BLK], F32, name="dqps",
                                        tag="dq")
                        for slot, i in enumerate(ig):
                            js = act_per_i[i]
                            for jn, j in enumerate(js):
                                p = pidx[(i, j)]
                                nc.tensor.matmul(
                                    dq[:, slot * BLK:(slot + 1) * BLK],
                                    tkns[h][:, j * DK:(j + 1) * DK],
                                    dstT[:, p * BLK:(p + 1) * BLK],
                                    start=(slot == 0 and jn == 0),
                                    stop=(slot == len(ig) - 1
                                          and jn == len(js) - 1),
                                    skip_group_check=True)
                        sb = outsb.tile([DK, 2 * BLK], F32, name="dqsb",
                                        tag="dq")
                        nc.scalar.copy(sb[:], dq[:])
                        for slot, i in enumerate(ig):
                            nc.sync.dma_start(
                                dQo[h, :, i * BLK:(i + 1) * BLK],
                                sb[:, slot * BLK:(slot + 1) * BLK])
    nc.compile()
    return nc, npair, pairs


_prog_cache = {}


def _get_prog(mask):
    key = tuple(int(x) for x in np.asarray(mask).astype(np.int64).ravel())
    if key not in _prog_cache:
        _prog_cache[key] = _build(key)
    return _prog_cache[key]


def kernel(q, k, v, dO, block_sparse_mask, _trace=False):
    q = np.ascontiguousarray(np.asarray(q, dtype=np.float32))
    k = np.ascontiguousarray(np.asarray(k, dtype=np.float32))
    v = np.ascontiguousarray(np.asarray(v, dtype=np.float32))
    dO = np.ascontiguousarray(np.asarray(dO, dtype=np.float32))
    mask = np.asarray(block_sparse_mask)

    nc, npair, pairs = _get_prog(mask)

    def tlay(x, g):  # head g of (1,N,D) -> [64, N] transposed bf16
        return np.ascontiguousarray(
            x[0, :, g * DK:(g + 1) * DK].T).astype(_BF)

    def nlay(x, g, scale):  # head g natural -> [128, T*DK]
        y = (x[0, :, g * DK:(g + 1) * DK] * scale).reshape(T, BLK, DK)
        return np.ascontiguousarray(
            y.transpose(1, 0, 2).reshape(BLK, T * DK)).astype(_BF)

    in_maps = []
    for c in range(NCORES):
        im = {}
        for h in range(HPC):
            g = c * HPC + h
            im[f"qdo{h}"] = np.ascontiguousarray(
                np.concatenate([tlay(q, g), tlay(dO, g)], axis=0))
            im[f"kv{h}"] = np.ascontiguousarray(
                np.concatenate([tlay(k, g), tlay(v, g)], axis=0))
            im[f"qns{h}"] = nlay(q, g, SCALE)
            im[f"kns{h}"] = nlay(k, g, SCALE)
            don = nlay(dO, g, 1.0).reshape(BLK, T, DK)
            im[f"donp{h}"] = np.ascontiguousarray(
                don[:, [i for (i, j) in pairs], :].reshape(BLK, npair * DK))
        in_maps.append(im)

    res = run_bass_kernel_spmd(nc, in_maps, list(range(NCORES)), trace=_trace)
    if _trace:
        kernel.last_exec_time_ns = res.exec_time_ns

    m64 = np.asarray(mask).astype(np.int64)
    empty_i = [i for i in range(T) if not m64[i, :].any()]
    empty_j = [j for j in range(T) if not m64[:, j].any()]

    dQ = np.empty((1, N, D), np.float32)
    dK = np.empty((1, N, D), np.float32)
    dV = np.empty((1, N, D), np.float32)
    for c in range(NCORES):
        r = res.results[c]
        for h in range(HPC):
            g = c * HPC + h
            dQ[0, :, g * DK:(g + 1) * DK] = r["dQo"][h].T
            dK[0, :, g * DK:(g + 1) * DK] = r["dKo"][h].T
            dV[0, :, g * DK:(g + 1) * DK] = r["dVo"][h].T
    for i in empty_i:
        dQ[0, i * BLK:(i + 1) * BLK, :] = 0.0
    for j in empty_j:
        dK[0, j * BLK:(j + 1) * BLK, :] = 0.0
        dV[0, j * BLK:(j + 1) * BLK, :] = 0.0
    return dQ, dK, dV
